# revision 9
# baseline (speedup 1.0000x reference)
"""Trainium2 Bass kernel for CenterAlignment (segment-reduce + EMA + normalize + loss).

Contract: kernel(**inputs) takes FULL unsharded numpy inputs
  x:          [65536, 1024] f32
  center_img: [1000, 1024]  f32
  center_skt: [1000, 1024]  f32
  l:          [32768]       int64
and returns the full scalar loss (f32, shape ()).

Strategy (8 NeuronCores, SPMD, class-partitioned):
  - Host prep (cheap, exact): crop pairs share a label, so x0+x1 is added
    on host (f32) and cast once to fp8 (the matmuls ran on fp8 operands in
    the data-parallel variant too, so no precision change). Per-class
    counts come from np.bincount (exact).
  - Classes are split into 8 contiguous groups with near-equal row counts
    (cuts at row-count quantiles). ALL rows of a class go to the one core
    that owns the class, so per-class sums complete locally and the kernel
    needs NO collectives. Each core's rows are padded with zero-rows to a
    fixed 4352 (=B/8 + slack; a zero row contributes nothing to any sum);
    each core's class window is <=128*n_chunks classes. kernel() picks
    n_chunks=1 when the windows allow (uniform labels give ~125-127 wide
    windows) and falls back to n_chunks=2; both variants are the same
    program parameterized.
  - Labels ship relative to the core's window base, so the device one-hot
    is only [128, 2, 128*n_chunks] fp8 per tile pair.
  - Per-class sums via fp8 DoubleRow matmuls: 17 tile-pairs x n_chunks x
    2 feature halves matmuls of [128,2,128]^T @ [128,2,512] accumulating
    into 2*n_chunks PSUM banks.
  - Tail per class chunk (EMA + normalize + masked loss) runs on f32 sums
    straight from PSUM (no drain, no bf16 round-trip):
    with S1=sum(upd^2), S12=sum((upd+cs)^2), S3=sum(cs^2),
    ||upd/||upd|| - cs||^2 = (1+S3) - (S12-S1-S3)/sqrt(S1).
    rec=0.1/max(cnt,1) and pres=min(cnt,1) ship from host (aux input).
  - Each core outputs [128, 2*n_chunks] = (masked loss, present) per
    chunk; the final sum + divide happens on host while unsharding.
"""

import sys

for _p in ("/opt/trn_rl_repo",):
    if _p not in sys.path:
        sys.path.insert(0, _p)

import numpy as np
import ml_dtypes

from concourse import bacc, bass, tile
from concourse import mybir
from concourse import bass_utils

f32 = mybir.dt.float32
f16 = mybir.dt.float16
bf16 = mybir.dt.bfloat16
fp8 = mybir.dt.float8e4
i32 = mybir.dt.int32

N_CORES = 8
B = 32768              # labels per batch (pair rows)
NUM_CROPS = 2
FEA = 1024             # feature dim
N_CLASSES = 1000
MOMENTUM = 0.9
# per-core padded row capacity: avg is 4096 (=B/8) but contiguous class
# groups can't all be exactly average; quantile cuts bound each group by
# 4096 + max-class-count (~60 for uniform labels), so 4352 (=17*256) has
# ample slack.
ROWS_PER_CORE = 4352


def build_program(rows_per_core: int = ROWS_PER_CORE, repeat: int = 1,
                  n_chunks: int = 1):
    """Build the SPMD Bass program (same graph on all 8 cores).

    n_chunks: per-core class window is 128*n_chunks classes.
    repeat: unroll the whole computation this many times (timing instrument:
      slope difference between repeat=R and repeat=1 isolates pure on-device
      time from dispatch overhead). kernel() always uses repeat=1.
    """
    assert rows_per_core % 256 == 0
    n_tiles = rows_per_core // 128
    n_pairs = n_tiles // 2
    cw = 128 * n_chunks

    nc = bacc.Bacc(
        "TRN2",
        target_bir_lowering=False,
        debug=False,
        enable_asserts=False,
        num_devices=N_CORES,
    )

    xq_d = nc.dram_tensor("xq", [rows_per_core, FEA], fp8, kind="ExternalInput")
    lab_d = nc.dram_tensor("labels", [rows_per_core], i32, kind="ExternalInput")
    ci_d = nc.dram_tensor("ci", [cw, FEA], bf16, kind="ExternalInput")
    cs_d = nc.dram_tensor("cs", [cw, FEA], bf16, kind="ExternalInput")
    aux_d = nc.dram_tensor("aux", [128, 2 * n_chunks], f32, kind="ExternalInput")
    out_d = nc.dram_tensor("loss", [128, 2 * n_chunks], f32, kind="ExternalOutput")

    # row r of this core's slice lives at partition r // n_tiles, tile
    # r % n_tiles (labels land contiguously per partition)
    xq_r = xq_d[:, :].rearrange("(p t) c -> p t c", p=128)

    Sq = mybir.ActivationFunctionType.Square

    with tile.TileContext(nc) as tc:
        with (
            tc.tile_pool(name="const", bufs=1) as const_pool,
            tc.tile_pool(name="oh", bufs=1) as oh_pool,
            tc.tile_pool(name="psum", bufs=1, space="PSUM") as psum_pool,
        ):
            def run_body():
                # ---- input loads ----
                lab_sb = const_pool.tile([128, n_tiles], i32, tag="lab32")
                nc.gpsimd.dma_start(
                    lab_sb[:], lab_d[:].rearrange("(p t) -> p t", p=128)
                )
                iota_t = const_pool.tile([128, cw], f16, tag="iota")
                nc.gpsimd.iota(
                    iota_t[:],
                    pattern=[[1, cw]],
                    base=0,
                    channel_multiplier=0,
                    allow_small_or_imprecise_dtypes=True,
                )
                labf = const_pool.tile([128, n_tiles], f32, tag="labf")
                nc.vector.tensor_copy(labf[:], lab_sb[:])

                # x: 34KB contiguous per partition -> four DMAs on two queues;
                # the last chunk is small so the final matmuls (which chase
                # the DMA) expose less work after the last byte lands
                xq_sb = const_pool.tile([128, n_tiles, FEA], fp8, tag="xq")
                qt = (n_tiles + 2) // 4 + 1
                bnds = [0, qt, 2 * qt, 3 * qt, n_tiles]
                for i in range(4):
                    eng = nc.sync if i % 2 == 0 else nc.gpsimd
                    eng.dma_start(
                        xq_sb[:, bnds[i]:bnds[i + 1], :],
                        xq_r[:, bnds[i]:bnds[i + 1], :],
                    )

                ci_sb = const_pool.tile([128, n_chunks, FEA], bf16, tag="ci")
                nc.scalar.dma_start(
                    ci_sb[:], ci_d[:, :].rearrange("(c p) f -> p c f", p=128)
                )
                cs_sb = const_pool.tile([128, n_chunks, FEA], bf16, tag="cs")
                nc.scalar.dma_start(
                    cs_sb[:], cs_d[:, :].rearrange("(c p) f -> p c f", p=128)
                )
                aux_sb = const_pool.tile([128, 2 * n_chunks], f32, tag="aux")
                nc.scalar.dma_start(aux_sb[:], aux_d[:, :])

                # pre-warm the ACT function tables used by the tail
                warm = const_pool.tile([1, 1], f32, tag="warm")
                warm2 = const_pool.tile([1, 1], f32, tag="warm2")
                nc.vector.memset(warm[:], 1.0)
                nc.scalar.activation(warm2[:], warm[:], Sq)
                nc.scalar.activation(
                    warm2[:], warm[:], mybir.ActivationFunctionType.Sqrt
                )

                # ---- one-hots: [128, 2, cw] fp8 per tile pair ----
                ohs = []
                for u in range(n_pairs):
                    ohp = oh_pool.tile([128, 2, cw], fp8, tag=f"ohp{u}",
                                       name=f"ohp{u}")
                    for jj in range(2):
                        t = 2 * u + jj
                        nc.vector.tensor_scalar(
                            ohp[:, jj, :],
                            iota_t[:],
                            labf[:, t : t + 1],
                            None,
                            op0=mybir.AluOpType.is_equal,
                        )
                    ohs.append(ohp)

                # ---- per-class sums: 2*n_chunks PSUM banks ----
                accs = [
                    [
                        psum_pool.tile([128, 512], f32, tag=f"acc{c}{h}",
                                       name=f"acc{c}{h}")
                        for h in range(2)
                    ]
                    for c in range(n_chunks)
                ]
                for u in range(n_pairs):
                    for c in range(n_chunks):
                        for h in range(2):
                            nc.tensor.matmul(
                                accs[c][h][:],
                                ohs[u][:, :, bass.ts(c, 128)],
                                xq_sb[:, 2 * u : 2 * u + 2, bass.ts(h, 512)],
                                perf_mode=mybir.MatmulPerfMode.DoubleRow,
                                start=(u == 0),
                                stop=(u == n_pairs - 1),
                            )

                # ---- tail per class chunk ----
                stack = const_pool.tile([128, 2 * n_chunks], f32, tag="stack")
                for c in range(n_chunks):
                    rec = aux_sb[:, 2 * c : 2 * c + 1]
                    pres = aux_sb[:, 2 * c + 1 : 2 * c + 2]

                    # S3 = sum(cs^2) per class
                    s3tmp = const_pool.tile([128, FEA], f32, tag="tailC")
                    s3 = const_pool.tile([128, 1], f32, tag=f"s3_{c}",
                                         name=f"s3_{c}")
                    nc.scalar.activation(s3tmp[:], cs_sb[:, c, :], Sq,
                                         accum_out=s3[:])

                    s1p = [None, None]
                    s12p = [None, None]
                    for h in range(2):
                        hc = bass.ts(h, 512)
                        # mean*(1-momentum) = sums * (0.1/count)
                        msc = const_pool.tile([128, 512], f32, tag="tailA")
                        nc.vector.tensor_scalar(
                            msc[:],
                            accs[c][h][:],
                            rec,
                            None,
                            op0=mybir.AluOpType.mult,
                        )
                        # upd = ci*momentum + mean*(1-momentum)
                        upd = const_pool.tile([128, 512], f32, tag="tailB")
                        nc.vector.scalar_tensor_tensor(
                            upd[:],
                            in0=ci_sb[:, c, hc],
                            scalar=MOMENTUM,
                            in1=msc[:],
                            op0=mybir.AluOpType.mult,
                            op1=mybir.AluOpType.add,
                        )
                        sqt = const_pool.tile([128, 512], f32, tag="tailC")
                        s1p[h] = const_pool.tile([128, 1], f32, tag=f"s1p{c}{h}",
                                                 name=f"s1p{c}{h}")
                        nc.scalar.activation(sqt[:], upd[:], Sq,
                                             accum_out=s1p[h][:])
                        ucs = const_pool.tile([128, 512], f32, tag="tailA")
                        nc.vector.tensor_tensor(
                            ucs[:], upd[:], cs_sb[:, c, hc],
                            op=mybir.AluOpType.add,
                        )
                        sqt2 = const_pool.tile([128, 512], f32, tag="tailB")
                        s12p[h] = const_pool.tile([128, 1], f32,
                                                  tag=f"s12p{c}{h}",
                                                  name=f"s12p{c}{h}")
                        nc.scalar.activation(sqt2[:], ucs[:], Sq,
                                             accum_out=s12p[h][:])

                    s1 = const_pool.tile([128, 1], f32, tag=f"s1_{c}",
                                         name=f"s1_{c}")
                    nc.vector.tensor_tensor(s1[:], s1p[0][:], s1p[1][:],
                                            op=mybir.AluOpType.add)
                    s12 = const_pool.tile([128, 1], f32, tag=f"s12_{c}",
                                          name=f"s12_{c}")
                    nc.vector.tensor_tensor(s12[:], s12p[0][:], s12p[1][:],
                                            op=mybir.AluOpType.add)

                    # per_cls = (1 + S3) - (S12 - S1 - S3) / sqrt(S1)
                    s3p1 = const_pool.tile([128, 1], f32, tag="s3p1")
                    nc.vector.tensor_scalar(
                        s3p1[:], s3[:], 1.0, None, op0=mybir.AluOpType.add
                    )
                    s1g = const_pool.tile([128, 1], f32, tag="s1g")
                    nc.vector.tensor_scalar_max(s1g[:], s1[:], 1e-30)
                    s1r = const_pool.tile([128, 1], f32, tag="s1r")
                    nc.vector.reciprocal(s1r[:], s1g[:])
                    rsq = const_pool.tile([128, 1], f32, tag="rsq")
                    nc.scalar.activation(
                        rsq[:], s1r[:], mybir.ActivationFunctionType.Sqrt
                    )
                    t0 = const_pool.tile([128, 1], f32, tag="t0")
                    nc.vector.tensor_tensor(t0[:], s12[:], s1[:],
                                            op=mybir.AluOpType.subtract)
                    t1 = const_pool.tile([128, 1], f32, tag="t1")
                    nc.vector.tensor_tensor(t1[:], t0[:], s3[:],
                                            op=mybir.AluOpType.subtract)
                    t2 = const_pool.tile([128, 1], f32, tag="t2")
                    nc.vector.tensor_tensor(t2[:], t1[:], rsq[:],
                                            op=mybir.AluOpType.mult)
                    per = const_pool.tile([128, 1], f32, tag="per")
                    nc.vector.tensor_tensor(per[:], s3p1[:], t2[:],
                                            op=mybir.AluOpType.subtract)
                    nc.vector.tensor_tensor(
                        stack[:, 2 * c : 2 * c + 1], per[:], pres,
                        op=mybir.AluOpType.mult,
                    )
                    nc.vector.tensor_copy(stack[:, 2 * c + 1 : 2 * c + 2], pres)
                nc.sync.dma_start(out_d[:, :], stack[:])

            for _rep in range(repeat):
                run_body()

    nc.compile()
    return nc


def plan_partition(l, rows_per_core=ROWS_PER_CORE):
    """Contiguous class partition into 8 groups at row-count quantiles.

    Returns (cuts, n_chunks): cuts has 9 entries; group k owns classes
    [cuts[k], cuts[k+1]). n_chunks is 1 when every group's class span fits
    in one 128-class window, else 2 (window capacity 256).
    """
    l = np.asarray(l)
    cnt = np.bincount(l, minlength=N_CLASSES)          # pair rows per class
    S = np.concatenate([[0], np.cumsum(cnt)])          # S[c] = rows before c
    target = l.shape[0] / N_CORES
    cuts = [int(np.searchsorted(S, k * target, side="left"))
            for k in range(N_CORES)] + [N_CLASSES]
    spans = [cuts[k + 1] - cuts[k] for k in range(N_CORES)]
    grows = [int(cnt[cuts[k]:cuts[k + 1]].sum()) for k in range(N_CORES)]
    if max(grows) > rows_per_core:
        # capacity fallback for pathological label distributions: grow the
        # padded per-core row budget (program is rebuilt for the new size)
        rows_per_core = -(-max(grows) // 256) * 256
    n_chunks = 1 if max(spans) <= 128 else 2
    assert max(spans) <= 128 * n_chunks, f"class window overflow: {spans}"
    return cuts, n_chunks, rows_per_core


def make_in_maps(x, center_img, center_skt, l, rows_per_core=ROWS_PER_CORE,
                 plan=None):
    """Host prep: pair-add + fp8 cast + class-partitioned shard."""
    n = x.shape[0] // NUM_CROPS
    x = np.asarray(x, dtype=np.float32)
    l = np.asarray(l).astype(np.int64)
    if plan is None:
        plan = plan_partition(l, rows_per_core)
    cuts, n_chunks, rows_per_core = plan
    cw = 128 * n_chunks

    xs = x[:n] + x[n:]
    xq = xs.astype(ml_dtypes.float8_e4m3)

    order = np.argsort(l, kind="stable")
    l_sorted = l[order]
    xq_sorted = xq[order]

    cnt = np.bincount(l, minlength=N_CLASSES)
    S = np.concatenate([[0], np.cumsum(cnt)])
    counts = 2.0 * cnt.astype(np.float64)               # both crops
    rec_full = (0.1 / np.maximum(counts, 1.0)).astype(np.float32)
    pres_full = np.minimum(counts, 1.0).astype(np.float32)

    in_maps = []
    for k in range(N_CORES):
        c0, c1 = cuts[k], cuts[k + 1]
        r0, r1 = int(S[c0]), int(S[c1])
        nrows = r1 - r0
        xqk = np.zeros((rows_per_core, FEA), ml_dtypes.float8_e4m3)
        xqk[:nrows] = xq_sorted[r0:r1]
        # zero-pad rows: label 0 with x=0 contributes nothing
        labk = np.zeros((rows_per_core,), np.int32)
        labk[:nrows] = (l_sorted[r0:r1] - c0).astype(np.int32)
        cik = np.zeros((cw, FEA), ml_dtypes.bfloat16)
        cik[: c1 - c0] = center_img[c0:c1].astype(ml_dtypes.bfloat16)
        csk = np.zeros((cw, FEA), ml_dtypes.bfloat16)
        csk[: c1 - c0] = center_skt[c0:c1].astype(ml_dtypes.bfloat16)
        recw = np.zeros((cw,), np.float32)
        recw[: c1 - c0] = rec_full[c0:c1]
        presw = np.zeros((cw,), np.float32)
        presw[: c1 - c0] = pres_full[c0:c1]
        auxk = np.zeros((128, 2 * n_chunks), np.float32)
        for c in range(n_chunks):
            auxk[:, 2 * c] = recw[128 * c : 128 * (c + 1)]
            auxk[:, 2 * c + 1] = presw[128 * c : 128 * (c + 1)]
        # device layout: row r of the core slice is partition r // n_tiles,
        # tile r % n_tiles -- i.e. plain C-order reshape [128, n_tiles]
        in_maps.append(
            {
                "xq": xqk,
                "labels": labk,
                "ci": cik,
                "cs": csk,
                "aux": auxk,
            }
        )
    return in_maps


def reduce_outputs(res):
    """Host-side unshard: combine per-core [128, 2*n_chunks] partials."""
    parts = np.stack(
        [np.asarray(res[c]["loss"], np.float64) for c in range(N_CORES)]
    )
    loss_sum = parts[:, :, 0::2].sum()
    n_present = parts[:, :, 1::2].sum()
    return np.float32(loss_sum / n_present)


_CACHED_NC = {}


def _get_nc(n_chunks=1, rows_per_core=ROWS_PER_CORE):
    key = (n_chunks, rows_per_core)
    if key not in _CACHED_NC:
        _CACHED_NC[key] = build_program(rows_per_core=rows_per_core,
                                        n_chunks=n_chunks)
    return _CACHED_NC[key]


def prepare(x, center_img, center_skt, l):
    """Shared entry for kernel() and test harnesses: plan the partition,
    build (or fetch) the right program variant, and build the in_maps."""
    plan = plan_partition(l)
    nc = _get_nc(plan[1], plan[2])
    in_maps = make_in_maps(x, center_img, center_skt, l, plan=plan)
    return nc, in_maps


def kernel(x, center_img, center_skt, l):
    nc, in_maps = prepare(x, center_img, center_skt, l)
    res = bass_utils.run_bass_kernel_spmd(nc, in_maps, core_ids=list(range(N_CORES)))
    return reduce_outputs(res.results).reshape(()).astype(np.float32)


# revision 15
# speedup vs baseline: 1.2340x; 1.2340x over previous
"""Trainium2 Bass kernel for CenterAlignment (segment-reduce + EMA + normalize + loss).

Contract: kernel(**inputs) takes FULL unsharded numpy inputs
  x:          [65536, 1024] f32
  center_img: [1000, 1024]  f32
  center_skt: [1000, 1024]  f32
  l:          [32768]       int64
and returns the full scalar loss (f32, shape ()).

Strategy (8 NeuronCores, SPMD, class-partitioned):
  - Host prep (cheap, exact): crop pairs share a label, so x0+x1 is added
    on host (f32) and cast once to fp8 (the matmuls ran on fp8 operands in
    the data-parallel variant too, so no precision change). Per-class
    counts come from np.bincount (exact).
  - Classes are split into 8 contiguous groups with near-equal row counts
    (cuts at row-count quantiles). ALL rows of a class go to the one core
    that owns the class, so per-class sums complete locally and the kernel
    needs NO collectives. Each core's rows are padded with zero-rows to a
    fixed 4352 (=B/8 + slack; a zero row contributes nothing to any sum);
    each core's class window is <=128*n_chunks classes. kernel() picks
    n_chunks=1 when the windows allow (uniform labels give ~125-127 wide
    windows) and falls back to n_chunks=2; both variants are the same
    program parameterized.
  - Labels ship relative to the core's window base, so the device one-hot
    is only [128, 2, 128*n_chunks] fp8 per tile pair.
  - Per-class sums via fp8 DoubleRow matmuls: 17 tile-pairs x n_chunks x
    2 feature halves matmuls of [128,2,128]^T @ [128,2,512] accumulating
    into 2*n_chunks PSUM banks.
  - Tail per class chunk (EMA + normalize + masked loss) runs on f32 sums
    straight from PSUM (no drain, no bf16 round-trip):
    with S1=sum(upd^2), S12=sum((upd+cs)^2), S3=sum(cs^2),
    ||upd/||upd|| - cs||^2 = (1+S3) - (S12-S1-S3)/sqrt(S1).
    rec=0.1/max(cnt,1) and pres=min(cnt,1) ship from host (aux input).
  - Each core outputs [128, 2*n_chunks] = (masked loss, present) per
    chunk; the final sum + divide happens on host while unsharding.
"""

import sys

for _p in ("/opt/trn_rl_repo",):
    if _p not in sys.path:
        sys.path.insert(0, _p)

import numpy as np
import ml_dtypes

from concourse import bacc, bass, tile
from concourse import mybir
from concourse import bass_utils

f32 = mybir.dt.float32
f16 = mybir.dt.float16
bf16 = mybir.dt.bfloat16
fp8 = mybir.dt.float8e4
i32 = mybir.dt.int32

N_CORES = 8
B = 32768              # labels per batch (pair rows)
NUM_CROPS = 2
FEA = 1024             # feature dim
N_CLASSES = 1000
MOMENTUM = 0.9
# per-core padded row capacity: avg is 4096 (=B/8) but contiguous class
# groups can't all be exactly average; quantile cuts bound each group by
# 4096 + max-class-count (~60 for uniform labels), so 4352 (=17*256) has
# ample slack.
ROWS_PER_CORE = 4352


def build_program(rows_per_core: int = ROWS_PER_CORE, repeat: int = 1,
                  n_chunks: int = 1, stage: str = "full"):
    """Build the SPMD Bass program (same graph on all 8 cores).

    n_chunks: per-core class window is 128*n_chunks classes.
    repeat: unroll the whole computation this many times (timing instrument:
      slope difference between repeat=R and repeat=1 isolates pure on-device
      time from dispatch overhead). kernel() always uses repeat=1.
    stage: ablation instrument - "dma" (loads only), "mm" (loads + matmuls),
      "full" (the real kernel). Non-full stages write junk output but keep
      every DMA/MM live via slice consumes. kernel() always uses "full".
    """
    assert rows_per_core % 256 == 0
    n_tiles = rows_per_core // 128
    n_pairs = n_tiles // 2
    cw = 128 * n_chunks

    nc = bacc.Bacc(
        "TRN2",
        target_bir_lowering=False,
        debug=False,
        enable_asserts=False,
        num_devices=N_CORES,
    )

    xq_d = nc.dram_tensor("xq", [rows_per_core, FEA], fp8, kind="ExternalInput")
    lab_d = nc.dram_tensor("labels", [rows_per_core], i32, kind="ExternalInput")
    ci_d = nc.dram_tensor("ci", [cw, FEA], bf16, kind="ExternalInput")
    cs_d = nc.dram_tensor("cs", [cw, FEA], bf16, kind="ExternalInput")
    aux_d = nc.dram_tensor("aux", [128, 2 * n_chunks], f32, kind="ExternalInput")
    out_d = nc.dram_tensor("loss", [128, 2 * n_chunks], f32, kind="ExternalOutput")

    # row r of this core's slice lives at partition r // n_tiles, tile
    # r % n_tiles (labels land contiguously per partition)
    xq_r = xq_d[:, :].rearrange("(p t) c -> p t c", p=128)

    Sq = mybir.ActivationFunctionType.Square

    with tile.TileContext(nc) as tc:
        with (
            tc.tile_pool(name="const", bufs=1) as const_pool,
            tc.tile_pool(name="lab", bufs=2) as lab_pool,
            tc.tile_pool(name="xqp", bufs=2) as xq_pool,
            tc.tile_pool(name="oh", bufs=2) as oh_pool,
            tc.tile_pool(name="out", bufs=2) as out_pool,
            tc.tile_pool(name="psum", bufs=2, space="PSUM") as psum_pool,
        ):
            # true constants: hoisted out of the iteration body
            iota_t = const_pool.tile([128, cw], f16, tag="iota")
            nc.gpsimd.iota(
                iota_t[:],
                pattern=[[1, cw]],
                base=0,
                channel_multiplier=0,
                allow_small_or_imprecise_dtypes=True,
            )
            # pre-warm the ACT function tables used by the tail
            warm = const_pool.tile([1, 1], f32, tag="warm")
            warm2 = const_pool.tile([1, 1], f32, tag="warm2")
            nc.vector.memset(warm[:], 1.0)
            nc.scalar.activation(warm2[:], warm[:],
                                 mybir.ActivationFunctionType.Square)
            nc.scalar.activation(
                warm2[:], warm[:], mybir.ActivationFunctionType.Sqrt
            )

            # per-call constant inputs: loaded once, overlap the x DMA
            ci_sb = const_pool.tile([128, n_chunks, FEA], bf16, tag="ci")
            nc.scalar.dma_start(
                ci_sb[:], ci_d[:, :].rearrange("(c p) f -> p c f", p=128)
            )
            cs_sb = const_pool.tile([128, n_chunks, FEA], bf16, tag="cs")
            nc.scalar.dma_start(
                cs_sb[:], cs_d[:, :].rearrange("(c p) f -> p c f", p=128)
            )
            aux_sb = const_pool.tile([128, 2 * n_chunks], f32, tag="aux")
            nc.scalar.dma_start(aux_sb[:], aux_d[:, :])

            def run_body():
                # ---- input loads ----
                lab_sb = lab_pool.tile([128, n_tiles], i32, tag="lab32")
                nc.gpsimd.dma_start(
                    lab_sb[:], lab_d[:].rearrange("(p t) -> p t", p=128)
                )
                labf = lab_pool.tile([128, n_tiles], f32, tag="labf")
                nc.vector.tensor_copy(labf[:], lab_sb[:])

                # x: 34KB contiguous per partition -> four DMAs on two queues;
                # the last chunk is small so the final matmuls (which chase
                # the DMA) expose less work after the last byte lands
                xq_sb = xq_pool.tile([128, n_tiles, FEA], fp8, tag="xq")
                qt = (n_tiles + 2) // 4 + 1
                bnds = [0, qt, 2 * qt, 3 * qt, n_tiles]
                for i in range(4):
                    eng = nc.sync if i % 2 == 0 else nc.gpsimd
                    eng.dma_start(
                        xq_sb[:, bnds[i]:bnds[i + 1], :],
                        xq_r[:, bnds[i]:bnds[i + 1], :],
                    )

                if stage == "dma":
                    # consume one slice per DMA so nothing is dead-code'd
                    cons = out_pool.tile([128, 2], f32, tag="cons")
                    nc.vector.tensor_copy(cons[:], labf[:, 0:2])
                    for i in range(4):
                        nc.vector.tensor_tensor(
                            cons[:], cons[:], xq_sb[:, bnds[i], 0:2],
                            op=mybir.AluOpType.add,
                        )
                    for src in (ci_sb[:, 0, 0:2], cs_sb[:, 0, 0:2],
                                aux_sb[:, 0:2]):
                        nc.vector.tensor_tensor(
                            cons[:], cons[:], src, op=mybir.AluOpType.add
                        )
                    nc.sync.dma_start(out_d[:, 0:2], cons[:])
                    return

                # ---- one-hots: [128, 2, cw] fp8 per tile pair ----
                ohs = []
                for u in range(n_pairs):
                    ohp = oh_pool.tile([128, 2, cw], fp8, tag=f"ohp{u}",
                                       name=f"ohp{u}")
                    for jj in range(2):
                        t = 2 * u + jj
                        nc.vector.tensor_scalar(
                            ohp[:, jj, :],
                            iota_t[:],
                            labf[:, t : t + 1],
                            None,
                            op0=mybir.AluOpType.is_equal,
                        )
                    ohs.append(ohp)

                # ---- per-class sums: 2*n_chunks PSUM banks ----
                accs = [
                    [
                        psum_pool.tile([128, 512], f32, tag=f"acc{c}{h}",
                                       name=f"acc{c}{h}")
                        for h in range(2)
                    ]
                    for c in range(n_chunks)
                ]
                for u in range(n_pairs):
                    for c in range(n_chunks):
                        for h in range(2):
                            nc.tensor.matmul(
                                accs[c][h][:],
                                ohs[u][:, :, bass.ts(c, 128)],
                                xq_sb[:, 2 * u : 2 * u + 2, bass.ts(h, 512)],
                                perf_mode=mybir.MatmulPerfMode.DoubleRow,
                                start=(u == 0),
                                stop=(u == n_pairs - 1),
                            )

                if stage == "mm":
                    cons = out_pool.tile([128, 2], f32, tag="cons")
                    nc.vector.tensor_copy(cons[:], aux_sb[:, 0:2])
                    for c in range(n_chunks):
                        for h in range(2):
                            nc.vector.tensor_tensor(
                                cons[:], cons[:], accs[c][h][:, 0:2],
                                op=mybir.AluOpType.add,
                            )
                    for src in (ci_sb[:, 0, 0:2], cs_sb[:, 0, 0:2]):
                        nc.vector.tensor_tensor(
                            cons[:], cons[:], src, op=mybir.AluOpType.add
                        )
                    nc.sync.dma_start(out_d[:, 0:2], cons[:])
                    return

                # ---- tail per class chunk ----
                stack = out_pool.tile([128, 2 * n_chunks], f32, tag="stack")
                for c in range(n_chunks):
                    rec = aux_sb[:, 2 * c : 2 * c + 1]
                    pres = aux_sb[:, 2 * c + 1 : 2 * c + 2]

                    # S3 = sum(cs^2) per class
                    s3tmp = const_pool.tile([128, FEA], f32, tag="tailC")
                    s3 = const_pool.tile([128, 1], f32, tag=f"s3_{c}",
                                         name=f"s3_{c}")
                    nc.scalar.activation(s3tmp[:], cs_sb[:, c, :], Sq,
                                         accum_out=s3[:])

                    s1p = [None, None]
                    s12p = [None, None]
                    for h in range(2):
                        hc = bass.ts(h, 512)
                        # mean*(1-momentum) = sums * (0.1/count)
                        msc = const_pool.tile([128, 512], f32, tag="tailA")
                        nc.vector.tensor_scalar(
                            msc[:],
                            accs[c][h][:],
                            rec,
                            None,
                            op0=mybir.AluOpType.mult,
                        )
                        # upd = ci*momentum + mean*(1-momentum)
                        upd = const_pool.tile([128, 512], f32, tag="tailB")
                        nc.vector.scalar_tensor_tensor(
                            upd[:],
                            in0=ci_sb[:, c, hc],
                            scalar=MOMENTUM,
                            in1=msc[:],
                            op0=mybir.AluOpType.mult,
                            op1=mybir.AluOpType.add,
                        )
                        sqt = const_pool.tile([128, 512], f32, tag="tailC")
                        s1p[h] = const_pool.tile([128, 1], f32, tag=f"s1p{c}{h}",
                                                 name=f"s1p{c}{h}")
                        nc.scalar.activation(sqt[:], upd[:], Sq,
                                             accum_out=s1p[h][:])
                        ucs = const_pool.tile([128, 512], f32, tag="tailA")
                        nc.vector.tensor_tensor(
                            ucs[:], upd[:], cs_sb[:, c, hc],
                            op=mybir.AluOpType.add,
                        )
                        sqt2 = const_pool.tile([128, 512], f32, tag="tailB")
                        s12p[h] = const_pool.tile([128, 1], f32,
                                                  tag=f"s12p{c}{h}",
                                                  name=f"s12p{c}{h}")
                        nc.scalar.activation(sqt2[:], ucs[:], Sq,
                                             accum_out=s12p[h][:])

                    s1 = const_pool.tile([128, 1], f32, tag=f"s1_{c}",
                                         name=f"s1_{c}")
                    nc.vector.tensor_tensor(s1[:], s1p[0][:], s1p[1][:],
                                            op=mybir.AluOpType.add)
                    s12 = const_pool.tile([128, 1], f32, tag=f"s12_{c}",
                                          name=f"s12_{c}")
                    nc.vector.tensor_tensor(s12[:], s12p[0][:], s12p[1][:],
                                            op=mybir.AluOpType.add)

                    # per_cls = (1 + S3) - (S12 - S1 - S3) / sqrt(S1)
                    s3p1 = const_pool.tile([128, 1], f32, tag="s3p1")
                    nc.vector.tensor_scalar(
                        s3p1[:], s3[:], 1.0, None, op0=mybir.AluOpType.add
                    )
                    s1g = const_pool.tile([128, 1], f32, tag="s1g")
                    nc.vector.tensor_scalar_max(s1g[:], s1[:], 1e-30)
                    s1r = const_pool.tile([128, 1], f32, tag="s1r")
                    nc.vector.reciprocal(s1r[:], s1g[:])
                    rsq = const_pool.tile([128, 1], f32, tag="rsq")
                    nc.scalar.activation(
                        rsq[:], s1r[:], mybir.ActivationFunctionType.Sqrt
                    )
                    t0 = const_pool.tile([128, 1], f32, tag="t0")
                    nc.vector.tensor_tensor(t0[:], s12[:], s1[:],
                                            op=mybir.AluOpType.subtract)
                    t1 = const_pool.tile([128, 1], f32, tag="t1")
                    nc.vector.tensor_tensor(t1[:], t0[:], s3[:],
                                            op=mybir.AluOpType.subtract)
                    t2 = const_pool.tile([128, 1], f32, tag="t2")
                    nc.vector.tensor_tensor(t2[:], t1[:], rsq[:],
                                            op=mybir.AluOpType.mult)
                    per = const_pool.tile([128, 1], f32, tag="per")
                    nc.vector.tensor_tensor(per[:], s3p1[:], t2[:],
                                            op=mybir.AluOpType.subtract)
                    nc.vector.tensor_tensor(
                        stack[:, 2 * c : 2 * c + 1], per[:], pres,
                        op=mybir.AluOpType.mult,
                    )
                    nc.vector.tensor_copy(stack[:, 2 * c + 1 : 2 * c + 2], pres)
                nc.sync.dma_start(out_d[:, :], stack[:])

            for _rep in range(repeat):
                run_body()

    nc.compile()
    return nc


def plan_partition(l, rows_per_core=ROWS_PER_CORE):
    """Contiguous class partition into 8 groups at row-count quantiles.

    Returns (cuts, n_chunks): cuts has 9 entries; group k owns classes
    [cuts[k], cuts[k+1]). n_chunks is 1 when every group's class span fits
    in one 128-class window, else 2 (window capacity 256).
    """
    l = np.asarray(l)
    cnt = np.bincount(l, minlength=N_CLASSES)          # pair rows per class
    S = np.concatenate([[0], np.cumsum(cnt)])          # S[c] = rows before c
    target = l.shape[0] / N_CORES
    cuts = [int(np.searchsorted(S, k * target, side="left"))
            for k in range(N_CORES)] + [N_CLASSES]
    spans = [cuts[k + 1] - cuts[k] for k in range(N_CORES)]
    grows = [int(cnt[cuts[k]:cuts[k + 1]].sum()) for k in range(N_CORES)]
    if max(grows) > rows_per_core:
        # capacity fallback for pathological label distributions: grow the
        # padded per-core row budget (program is rebuilt for the new size)
        rows_per_core = -(-max(grows) // 256) * 256
    n_chunks = 1 if max(spans) <= 128 else 2
    assert max(spans) <= 128 * n_chunks, f"class window overflow: {spans}"
    return cuts, n_chunks, rows_per_core


def make_in_maps(x, center_img, center_skt, l, rows_per_core=ROWS_PER_CORE,
                 plan=None):
    """Host prep: pair-add + fp8 cast + class-partitioned shard."""
    n = x.shape[0] // NUM_CROPS
    x = np.asarray(x, dtype=np.float32)
    l = np.asarray(l).astype(np.int64)
    if plan is None:
        plan = plan_partition(l, rows_per_core)
    cuts, n_chunks, rows_per_core = plan
    cw = 128 * n_chunks

    xs = x[:n] + x[n:]
    xq = xs.astype(ml_dtypes.float8_e4m3)

    order = np.argsort(l, kind="stable")
    l_sorted = l[order]
    xq_sorted = xq[order]

    cnt = np.bincount(l, minlength=N_CLASSES)
    S = np.concatenate([[0], np.cumsum(cnt)])
    counts = 2.0 * cnt.astype(np.float64)               # both crops
    rec_full = (0.1 / np.maximum(counts, 1.0)).astype(np.float32)
    pres_full = np.minimum(counts, 1.0).astype(np.float32)

    in_maps = []
    for k in range(N_CORES):
        c0, c1 = cuts[k], cuts[k + 1]
        r0, r1 = int(S[c0]), int(S[c1])
        nrows = r1 - r0
        xqk = np.zeros((rows_per_core, FEA), ml_dtypes.float8_e4m3)
        xqk[:nrows] = xq_sorted[r0:r1]
        # zero-pad rows: label 0 with x=0 contributes nothing
        labk = np.zeros((rows_per_core,), np.int32)
        labk[:nrows] = (l_sorted[r0:r1] - c0).astype(np.int32)
        cik = np.zeros((cw, FEA), ml_dtypes.bfloat16)
        cik[: c1 - c0] = center_img[c0:c1].astype(ml_dtypes.bfloat16)
        csk = np.zeros((cw, FEA), ml_dtypes.bfloat16)
        csk[: c1 - c0] = center_skt[c0:c1].astype(ml_dtypes.bfloat16)
        recw = np.zeros((cw,), np.float32)
        recw[: c1 - c0] = rec_full[c0:c1]
        presw = np.zeros((cw,), np.float32)
        presw[: c1 - c0] = pres_full[c0:c1]
        auxk = np.zeros((128, 2 * n_chunks), np.float32)
        for c in range(n_chunks):
            auxk[:, 2 * c] = recw[128 * c : 128 * (c + 1)]
            auxk[:, 2 * c + 1] = presw[128 * c : 128 * (c + 1)]
        # device layout: row r of the core slice is partition r // n_tiles,
        # tile r % n_tiles -- i.e. plain C-order reshape [128, n_tiles]
        in_maps.append(
            {
                "xq": xqk,
                "labels": labk,
                "ci": cik,
                "cs": csk,
                "aux": auxk,
            }
        )
    return in_maps


def reduce_outputs(res):
    """Host-side unshard: combine per-core [128, 2*n_chunks] partials."""
    parts = np.stack(
        [np.asarray(res[c]["loss"], np.float64) for c in range(N_CORES)]
    )
    loss_sum = parts[:, :, 0::2].sum()
    n_present = parts[:, :, 1::2].sum()
    return np.float32(loss_sum / n_present)


_CACHED_NC = {}


def _get_nc(n_chunks=1, rows_per_core=ROWS_PER_CORE):
    key = (n_chunks, rows_per_core)
    if key not in _CACHED_NC:
        _CACHED_NC[key] = build_program(rows_per_core=rows_per_core,
                                        n_chunks=n_chunks)
    return _CACHED_NC[key]


def prepare(x, center_img, center_skt, l):
    """Shared entry for kernel() and test harnesses: plan the partition,
    build (or fetch) the right program variant, and build the in_maps."""
    plan = plan_partition(l)
    nc = _get_nc(plan[1], plan[2])
    in_maps = make_in_maps(x, center_img, center_skt, l, plan=plan)
    return nc, in_maps


def kernel(x, center_img, center_skt, l):
    nc, in_maps = prepare(x, center_img, center_skt, l)
    res = bass_utils.run_bass_kernel_spmd(nc, in_maps, core_ids=list(range(N_CORES)))
    return reduce_outputs(res.results).reshape(()).astype(np.float32)


# revision 19
# speedup vs baseline: 1.3255x; 1.0742x over previous
"""Trainium2 Bass kernel for CenterAlignment (segment-reduce + EMA + normalize + loss).

Contract: kernel(**inputs) takes FULL unsharded numpy inputs
  x:          [65536, 1024] f32
  center_img: [1000, 1024]  f32
  center_skt: [1000, 1024]  f32
  l:          [32768]       int64
and returns the full scalar loss (f32, shape ()).

Strategy (8 NeuronCores, SPMD, class-partitioned):
  - Host prep (cheap, exact): crop pairs share a label, so x0+x1 is added
    on host (f32) and cast once to fp8 (the matmuls ran on fp8 operands in
    the data-parallel variant too, so no precision change). Per-class
    counts come from np.bincount (exact).
  - Classes are split into 8 contiguous groups with near-equal row counts
    (cuts at row-count quantiles). ALL rows of a class go to the one core
    that owns the class, so per-class sums complete locally and the kernel
    needs NO collectives. Each core's rows are padded with zero-rows to a
    fixed 4352 (=B/8 + slack; a zero row contributes nothing to any sum);
    each core's class window is <=128*n_chunks classes. kernel() picks
    n_chunks=1 when the windows allow (uniform labels give ~125-127 wide
    windows) and falls back to n_chunks=2; both variants are the same
    program parameterized.
  - Labels ship relative to the core's window base, so the device one-hot
    is only [128, 2, 128*n_chunks] fp8 per tile pair.
  - Per-class sums via fp8 DoubleRow matmuls: 17 tile-pairs x n_chunks x
    2 feature halves matmuls of [128,2,128]^T @ [128,2,512] accumulating
    into 2*n_chunks PSUM banks.
  - Tail per class chunk (EMA + normalize + masked loss) runs on f32 sums
    straight from PSUM (no drain, no bf16 round-trip):
    with S1=sum(upd^2), S12=sum((upd+cs)^2), S3=sum(cs^2),
    ||upd/||upd|| - cs||^2 = (1+S3) - (S12-S1-S3)/sqrt(S1).
    rec=0.1/max(cnt,1) and pres=min(cnt,1) ship from host (aux input).
  - Each core outputs [128, 2*n_chunks] = (masked loss, present) per
    chunk; the final sum + divide happens on host while unsharding.
"""

import sys

for _p in ("/opt/trn_rl_repo",):
    if _p not in sys.path:
        sys.path.insert(0, _p)

import numpy as np
import ml_dtypes

from concourse import bacc, bass, tile
from concourse import mybir
from concourse import bass_utils

f32 = mybir.dt.float32
f16 = mybir.dt.float16
bf16 = mybir.dt.bfloat16
fp8 = mybir.dt.float8e4
i32 = mybir.dt.int32

N_CORES = 8
B = 32768              # labels per batch (pair rows)
NUM_CROPS = 2
FEA = 1024             # feature dim
N_CLASSES = 1000
MOMENTUM = 0.9
# per-core padded row capacity: avg is 4096 (=B/8) but contiguous class
# groups can't all be exactly average; quantile cuts bound each group by
# 4096 + max-class-count (~60 for uniform labels), so 4352 (=17*256) has
# ample slack.
ROWS_PER_CORE = 4352


def build_program(rows_per_core: int = ROWS_PER_CORE, repeat: int = 1,
                  n_chunks: int = 1, stage: str = "full",
                  dma_mode: str = "4sa"):
    """Build the SPMD Bass program (same graph on all 8 cores).

    n_chunks: per-core class window is 128*n_chunks classes.
    repeat: unroll the whole computation this many times (timing instrument:
      slope difference between repeat=R and repeat=1 isolates pure on-device
      time from dispatch overhead). kernel() always uses repeat=1.
    stage: ablation instrument - "dma" (loads only), "mm" (loads + matmuls),
      "full" (the real kernel). Non-full stages write junk output but keep
      every DMA/MM live via slice consumes. kernel() always uses "full".
    dma_mode: how the x load is chunked across queues - "<n><queues>" with
      n chunks (last one small) round-robined over queues s=sync, a=scalar
      (HWDGE), g=gpsimd (SWDGE). e.g. "4sg", "4sa", "2sa", "1s".
    """
    assert rows_per_core % 256 == 0
    n_tiles = rows_per_core // 128
    n_pairs = n_tiles // 2
    cw = 128 * n_chunks

    nc = bacc.Bacc(
        "TRN2",
        target_bir_lowering=False,
        debug=False,
        enable_asserts=False,
        num_devices=N_CORES,
    )

    xq_d = nc.dram_tensor("xq", [rows_per_core, FEA], fp8, kind="ExternalInput")
    lab_d = nc.dram_tensor("labels", [rows_per_core], i32, kind="ExternalInput")
    ci_d = nc.dram_tensor("ci", [cw, FEA], bf16, kind="ExternalInput")
    cs_d = nc.dram_tensor("cs", [cw, FEA], bf16, kind="ExternalInput")
    aux_d = nc.dram_tensor("aux", [128, 2 * n_chunks], f32, kind="ExternalInput")
    out_d = nc.dram_tensor("loss", [128, 2 * n_chunks], f32, kind="ExternalOutput")

    # row r of this core's slice lives at partition r // n_tiles, tile
    # r % n_tiles (labels land contiguously per partition)
    xq_r = xq_d[:, :].rearrange("(p t) c -> p t c", p=128)

    Sq = mybir.ActivationFunctionType.Square

    with tile.TileContext(nc) as tc:
        with (
            tc.tile_pool(name="const", bufs=1) as const_pool,
            tc.tile_pool(name="lab", bufs=2) as lab_pool,
            tc.tile_pool(name="xqp", bufs=2) as xq_pool,
            tc.tile_pool(name="oh", bufs=2) as oh_pool,
            tc.tile_pool(name="out", bufs=2) as out_pool,
            tc.tile_pool(name="psum", bufs=2, space="PSUM") as psum_pool,
        ):
            # true constants: hoisted out of the iteration body
            iota_t = const_pool.tile([128, cw], f16, tag="iota")
            nc.gpsimd.iota(
                iota_t[:],
                pattern=[[1, cw]],
                base=0,
                channel_multiplier=0,
                allow_small_or_imprecise_dtypes=True,
            )
            # pre-warm the ACT function tables used by the tail
            warm = const_pool.tile([1, 1], f32, tag="warm")
            warm2 = const_pool.tile([1, 1], f32, tag="warm2")
            nc.vector.memset(warm[:], 1.0)
            nc.scalar.activation(warm2[:], warm[:],
                                 mybir.ActivationFunctionType.Square)
            nc.scalar.activation(
                warm2[:], warm[:], mybir.ActivationFunctionType.Sqrt
            )

            # per-call constant inputs: loaded once, overlap the x DMA
            ci_sb = const_pool.tile([128, n_chunks, FEA], bf16, tag="ci")
            nc.scalar.dma_start(
                ci_sb[:], ci_d[:, :].rearrange("(c p) f -> p c f", p=128)
            )
            cs_sb = const_pool.tile([128, n_chunks, FEA], bf16, tag="cs")
            nc.scalar.dma_start(
                cs_sb[:], cs_d[:, :].rearrange("(c p) f -> p c f", p=128)
            )
            aux_sb = const_pool.tile([128, 2 * n_chunks], f32, tag="aux")
            nc.scalar.dma_start(aux_sb[:], aux_d[:, :])

            # S3 = sum(cs^2) and 1+S3 are per-call constants too
            s3s, s3p1s = [], []
            for c in range(n_chunks):
                s3tmp = const_pool.tile([128, FEA], f32, tag="s3tmp")
                s3 = const_pool.tile([128, 1], f32, tag=f"s3_{c}",
                                     name=f"s3_{c}")
                nc.scalar.activation(s3tmp[:], cs_sb[:, c, :], Sq,
                                     accum_out=s3[:])
                s3p1 = const_pool.tile([128, 1], f32, tag=f"s3p1_{c}",
                                       name=f"s3p1_{c}")
                nc.vector.tensor_scalar(
                    s3p1[:], s3[:], 1.0, None, op0=mybir.AluOpType.add
                )
                s3s.append(s3)
                s3p1s.append(s3p1)

            def run_body():
                # ---- input loads ----
                lab_sb = lab_pool.tile([128, n_tiles], i32, tag="lab32")
                nc.gpsimd.dma_start(
                    lab_sb[:], lab_d[:].rearrange("(p t) -> p t", p=128)
                )
                labf = lab_pool.tile([128, n_tiles], f32, tag="labf")
                nc.vector.tensor_copy(labf[:], lab_sb[:])

                # x: 34KB contiguous per partition -> four DMAs on two queues;
                # the last chunk is small so the final matmuls (which chase
                # the DMA) expose less work after the last byte lands
                xq_sb = xq_pool.tile([128, n_tiles, FEA], fp8, tag="xq")
                n_dma = int(dma_mode[0])
                engs = {"s": nc.sync, "a": nc.scalar, "g": nc.gpsimd}
                qs = [engs[ch] for ch in dma_mode[1:]]
                if n_dma == 1:
                    bnds = [0, n_tiles]
                else:
                    qt = (n_tiles + n_dma - 2) // n_dma + 1
                    bnds = [min(i * qt, n_tiles) for i in range(n_dma)]
                    bnds.append(n_tiles)
                for i in range(n_dma):
                    if bnds[i] == bnds[i + 1]:
                        continue
                    qs[i % len(qs)].dma_start(
                        xq_sb[:, bnds[i]:bnds[i + 1], :],
                        xq_r[:, bnds[i]:bnds[i + 1], :],
                    )

                if stage == "dma":
                    # consume one slice per DMA so nothing is dead-code'd
                    cons = out_pool.tile([128, 2], f32, tag="cons")
                    nc.vector.tensor_copy(cons[:], labf[:, 0:2])
                    for b in bnds[:-1]:
                        nc.vector.tensor_tensor(
                            cons[:], cons[:], xq_sb[:, b, 0:2],
                            op=mybir.AluOpType.add,
                        )
                    for src in (ci_sb[:, 0, 0:2], cs_sb[:, 0, 0:2],
                                aux_sb[:, 0:2]):
                        nc.vector.tensor_tensor(
                            cons[:], cons[:], src, op=mybir.AluOpType.add
                        )
                    nc.sync.dma_start(out_d[:, 0:2], cons[:])
                    return

                # ---- one-hots: [128, 2, cw] fp8 per tile pair ----
                ohs = []
                for u in range(n_pairs):
                    ohp = oh_pool.tile([128, 2, cw], fp8, tag=f"ohp{u}",
                                       name=f"ohp{u}")
                    for jj in range(2):
                        t = 2 * u + jj
                        nc.vector.tensor_scalar(
                            ohp[:, jj, :],
                            iota_t[:],
                            labf[:, t : t + 1],
                            None,
                            op0=mybir.AluOpType.is_equal,
                        )
                    ohs.append(ohp)

                # ---- per-class sums: 2*n_chunks PSUM banks ----
                accs = [
                    [
                        psum_pool.tile([128, 512], f32, tag=f"acc{c}{h}",
                                       name=f"acc{c}{h}")
                        for h in range(2)
                    ]
                    for c in range(n_chunks)
                ]
                for u in range(n_pairs):
                    for c in range(n_chunks):
                        for h in range(2):
                            nc.tensor.matmul(
                                accs[c][h][:],
                                ohs[u][:, :, bass.ts(c, 128)],
                                xq_sb[:, 2 * u : 2 * u + 2, bass.ts(h, 512)],
                                perf_mode=mybir.MatmulPerfMode.DoubleRow,
                                start=(u == 0),
                                stop=(u == n_pairs - 1),
                            )

                if stage == "mm":
                    cons = out_pool.tile([128, 2], f32, tag="cons")
                    nc.vector.tensor_copy(cons[:], aux_sb[:, 0:2])
                    for c in range(n_chunks):
                        for h in range(2):
                            nc.vector.tensor_tensor(
                                cons[:], cons[:], accs[c][h][:, 0:2],
                                op=mybir.AluOpType.add,
                            )
                    for src in (ci_sb[:, 0, 0:2], cs_sb[:, 0, 0:2]):
                        nc.vector.tensor_tensor(
                            cons[:], cons[:], src, op=mybir.AluOpType.add
                        )
                    nc.sync.dma_start(out_d[:, 0:2], cons[:])
                    return

                # ---- tail per class chunk ----
                stack = out_pool.tile([128, 2 * n_chunks], f32, tag="stack")
                for c in range(n_chunks):
                    rec = aux_sb[:, 2 * c : 2 * c + 1]
                    pres = aux_sb[:, 2 * c + 1 : 2 * c + 2]

                    s3 = s3s[c]
                    s3p1 = s3p1s[c]

                    s1p = [None, None]
                    s12p = [None, None]
                    for h in range(2):
                        hc = bass.ts(h, 512)
                        # mean*(1-momentum) = sums * (0.1/count)
                        msc = const_pool.tile([128, 512], f32, tag="tailA")
                        nc.vector.tensor_scalar(
                            msc[:],
                            accs[c][h][:],
                            rec,
                            None,
                            op0=mybir.AluOpType.mult,
                        )
                        # upd = ci*momentum + mean*(1-momentum)
                        upd = const_pool.tile([128, 512], f32, tag="tailB")
                        nc.vector.scalar_tensor_tensor(
                            upd[:],
                            in0=ci_sb[:, c, hc],
                            scalar=MOMENTUM,
                            in1=msc[:],
                            op0=mybir.AluOpType.mult,
                            op1=mybir.AluOpType.add,
                        )
                        sqt = const_pool.tile([128, 512], f32, tag="tailC")
                        s1p[h] = const_pool.tile([128, 1], f32, tag=f"s1p{c}{h}",
                                                 name=f"s1p{c}{h}")
                        nc.scalar.activation(sqt[:], upd[:], Sq,
                                             accum_out=s1p[h][:])
                        ucs = const_pool.tile([128, 512], f32, tag="tailA")
                        nc.vector.tensor_tensor(
                            ucs[:], upd[:], cs_sb[:, c, hc],
                            op=mybir.AluOpType.add,
                        )
                        sqt2 = const_pool.tile([128, 512], f32, tag="tailB")
                        s12p[h] = const_pool.tile([128, 1], f32,
                                                  tag=f"s12p{c}{h}",
                                                  name=f"s12p{c}{h}")
                        nc.scalar.activation(sqt2[:], ucs[:], Sq,
                                             accum_out=s12p[h][:])

                    s1 = const_pool.tile([128, 1], f32, tag=f"s1_{c}",
                                         name=f"s1_{c}")
                    nc.vector.tensor_tensor(s1[:], s1p[0][:], s1p[1][:],
                                            op=mybir.AluOpType.add)
                    s12 = const_pool.tile([128, 1], f32, tag=f"s12_{c}",
                                          name=f"s12_{c}")
                    nc.vector.tensor_tensor(s12[:], s12p[0][:], s12p[1][:],
                                            op=mybir.AluOpType.add)

                    # per_cls = (1 + S3) - (S12 - S1 - S3) / sqrt(S1)
                    s1g = const_pool.tile([128, 1], f32, tag="s1g")
                    nc.vector.tensor_scalar_max(s1g[:], s1[:], 1e-30)
                    s1r = const_pool.tile([128, 1], f32, tag="s1r")
                    nc.vector.reciprocal(s1r[:], s1g[:])
                    rsq = const_pool.tile([128, 1], f32, tag="rsq")
                    nc.scalar.activation(
                        rsq[:], s1r[:], mybir.ActivationFunctionType.Sqrt
                    )
                    t0 = const_pool.tile([128, 1], f32, tag="t0")
                    nc.vector.tensor_tensor(t0[:], s12[:], s1[:],
                                            op=mybir.AluOpType.subtract)
                    t1 = const_pool.tile([128, 1], f32, tag="t1")
                    nc.vector.tensor_tensor(t1[:], t0[:], s3[:],
                                            op=mybir.AluOpType.subtract)
                    t2 = const_pool.tile([128, 1], f32, tag="t2")
                    nc.vector.tensor_tensor(t2[:], t1[:], rsq[:],
                                            op=mybir.AluOpType.mult)
                    per = const_pool.tile([128, 1], f32, tag="per")
                    nc.vector.tensor_tensor(per[:], s3p1[:], t2[:],
                                            op=mybir.AluOpType.subtract)
                    nc.vector.tensor_tensor(
                        stack[:, 2 * c : 2 * c + 1], per[:], pres,
                        op=mybir.AluOpType.mult,
                    )
                    nc.vector.tensor_copy(stack[:, 2 * c + 1 : 2 * c + 2], pres)
                nc.sync.dma_start(out_d[:, :], stack[:])

            for _rep in range(repeat):
                run_body()

    nc.compile()
    return nc


def plan_partition(l, rows_per_core=ROWS_PER_CORE):
    """Contiguous class partition into 8 groups at row-count quantiles.

    Returns (cuts, n_chunks): cuts has 9 entries; group k owns classes
    [cuts[k], cuts[k+1]). n_chunks is 1 when every group's class span fits
    in one 128-class window, else 2 (window capacity 256).
    """
    l = np.asarray(l)
    cnt = np.bincount(l, minlength=N_CLASSES)          # pair rows per class
    S = np.concatenate([[0], np.cumsum(cnt)])          # S[c] = rows before c
    target = l.shape[0] / N_CORES
    cuts = [int(np.searchsorted(S, k * target, side="left"))
            for k in range(N_CORES)] + [N_CLASSES]
    spans = [cuts[k + 1] - cuts[k] for k in range(N_CORES)]
    grows = [int(cnt[cuts[k]:cuts[k + 1]].sum()) for k in range(N_CORES)]
    if max(grows) > rows_per_core:
        # capacity fallback for pathological label distributions: grow the
        # padded per-core row budget (program is rebuilt for the new size)
        rows_per_core = -(-max(grows) // 256) * 256
    n_chunks = 1 if max(spans) <= 128 else 2
    assert max(spans) <= 128 * n_chunks, f"class window overflow: {spans}"
    return cuts, n_chunks, rows_per_core


def make_in_maps(x, center_img, center_skt, l, rows_per_core=ROWS_PER_CORE,
                 plan=None):
    """Host prep: pair-add + fp8 cast + class-partitioned shard."""
    n = x.shape[0] // NUM_CROPS
    x = np.asarray(x, dtype=np.float32)
    l = np.asarray(l).astype(np.int64)
    if plan is None:
        plan = plan_partition(l, rows_per_core)
    cuts, n_chunks, rows_per_core = plan
    cw = 128 * n_chunks

    xs = x[:n] + x[n:]
    xq = xs.astype(ml_dtypes.float8_e4m3)

    order = np.argsort(l, kind="stable")
    l_sorted = l[order]
    xq_sorted = xq[order]

    cnt = np.bincount(l, minlength=N_CLASSES)
    S = np.concatenate([[0], np.cumsum(cnt)])
    counts = 2.0 * cnt.astype(np.float64)               # both crops
    rec_full = (0.1 / np.maximum(counts, 1.0)).astype(np.float32)
    pres_full = np.minimum(counts, 1.0).astype(np.float32)

    in_maps = []
    for k in range(N_CORES):
        c0, c1 = cuts[k], cuts[k + 1]
        r0, r1 = int(S[c0]), int(S[c1])
        nrows = r1 - r0
        xqk = np.zeros((rows_per_core, FEA), ml_dtypes.float8_e4m3)
        xqk[:nrows] = xq_sorted[r0:r1]
        # zero-pad rows: label 0 with x=0 contributes nothing
        labk = np.zeros((rows_per_core,), np.int32)
        labk[:nrows] = (l_sorted[r0:r1] - c0).astype(np.int32)
        cik = np.zeros((cw, FEA), ml_dtypes.bfloat16)
        cik[: c1 - c0] = center_img[c0:c1].astype(ml_dtypes.bfloat16)
        csk = np.zeros((cw, FEA), ml_dtypes.bfloat16)
        csk[: c1 - c0] = center_skt[c0:c1].astype(ml_dtypes.bfloat16)
        recw = np.zeros((cw,), np.float32)
        recw[: c1 - c0] = rec_full[c0:c1]
        presw = np.zeros((cw,), np.float32)
        presw[: c1 - c0] = pres_full[c0:c1]
        auxk = np.zeros((128, 2 * n_chunks), np.float32)
        for c in range(n_chunks):
            auxk[:, 2 * c] = recw[128 * c : 128 * (c + 1)]
            auxk[:, 2 * c + 1] = presw[128 * c : 128 * (c + 1)]
        # device layout: row r of the core slice is partition r // n_tiles,
        # tile r % n_tiles -- i.e. plain C-order reshape [128, n_tiles]
        in_maps.append(
            {
                "xq": xqk,
                "labels": labk,
                "ci": cik,
                "cs": csk,
                "aux": auxk,
            }
        )
    return in_maps


def reduce_outputs(res):
    """Host-side unshard: combine per-core [128, 2*n_chunks] partials."""
    parts = np.stack(
        [np.asarray(res[c]["loss"], np.float64) for c in range(N_CORES)]
    )
    loss_sum = parts[:, :, 0::2].sum()
    n_present = parts[:, :, 1::2].sum()
    return np.float32(loss_sum / n_present)


_CACHED_NC = {}


def _get_nc(n_chunks=1, rows_per_core=ROWS_PER_CORE):
    key = (n_chunks, rows_per_core)
    if key not in _CACHED_NC:
        _CACHED_NC[key] = build_program(rows_per_core=rows_per_core,
                                        n_chunks=n_chunks)
    return _CACHED_NC[key]


def prepare(x, center_img, center_skt, l):
    """Shared entry for kernel() and test harnesses: plan the partition,
    build (or fetch) the right program variant, and build the in_maps."""
    plan = plan_partition(l)
    nc = _get_nc(plan[1], plan[2])
    in_maps = make_in_maps(x, center_img, center_skt, l, plan=plan)
    return nc, in_maps


def kernel(x, center_img, center_skt, l):
    nc, in_maps = prepare(x, center_img, center_skt, l)
    res = bass_utils.run_bass_kernel_spmd(nc, in_maps, core_ids=list(range(N_CORES)))
    return reduce_outputs(res.results).reshape(()).astype(np.float32)
